# revision 5
# baseline (speedup 1.0000x reference)
"""AttGNN (4x GCNConv + attention readout) on 8 Trainium2 NeuronCores.

Strategy (graph/data parallel, per sharding hint):
- Nodes sharded 8 ways by destination: core c owns global nodes
  [c*6250, (c+1)*6250), padded to 6272 rows (49 tiles of 128).
- Per layer: each core computes its local transform hws = (h @ W) * dinv
  (dinv folded at the source side), AllGather of the bf16 hws table
  [50176, 256], then per-edge message passing for its dst shard:
  indirect-DMA row gathers from the table + one-hot selection-matrix
  matmuls accumulating segment sums in PSUM (edges pre-sorted by dst,
  chunked 128 per matmul).  norm = dinv[src]*dinv[dst] is applied as
  dinv[src] in the table and dinv[dst] on the accumulated sum.
- Attention readout + output projection run locally on the dst shard.

Messages travel in bf16 (table/gather/selection matmul, fp32 PSUM
accumulate); dense transforms, attention and outputs are fp32.
"""
import numpy as np

P = 128
F = 256
L = 4
C = 40
N = 50000
E = 800000
NCORES = 8
OWN = N // NCORES          # 6250 nodes owned per core
SHARD = 6272               # padded shard rows (49 tiles)
NT = SHARD // P            # 49
NPAD = SHARD * NCORES      # 50176 table rows

_CACHE = {}


def _host_prep(edge_index):
    """Sort/shard edges, build per-core per-tile chunk metadata."""
    src = edge_index[0].astype(np.int64)
    dst = edge_index[1].astype(np.int64)
    # self loops
    loop = np.arange(N, dtype=np.int64)
    src = np.concatenate([src, loop])
    dst = np.concatenate([dst, loop])

    deg = np.bincount(dst, minlength=N).astype(np.float32)
    dinv = np.where(deg > 0, 1.0 / np.sqrt(deg), 0.0).astype(np.float32)

    # global node g -> table row (pad-aware)
    def trow(g):
        return (g // OWN) * SHARD + (g % OWN)

    srow = trow(src)
    dcore = dst // OWN
    dslot = dst % OWN

    # per (core, tile) edge lists
    order = np.argsort(dcore * OWN + dslot, kind="stable")
    srow, dslot, dcore = srow[order], dslot[order], dcore[order]

    counts = np.zeros((NCORES, NT), np.int64)
    per_ct_src = {}
    per_ct_dl = {}
    for c in range(NCORES):
        lo, hi = np.searchsorted(dcore, [c, c + 1])
        s_c, d_c = srow[lo:hi], dslot[lo:hi]
        for t in range(NT):
            a, b = np.searchsorted(d_c, [t * P, (t + 1) * P])
            counts[c, t] = b - a
            per_ct_src[(c, t)] = s_c[a:b]
            per_ct_dl[(c, t)] = (d_c[a:b] - t * P).astype(np.float32)

    nch = np.maximum(1, -(-counts.max(axis=0) // P))  # per-tile chunks, shared
    totc = int(nch.sum())

    ZROW = NPAD - 1  # a guaranteed all-zero table row (pad row of core 7)
    idxT = np.full((NCORES, P, totc), ZROW, np.int32)
    dstl = np.full((NCORES, P, totc), -1.0, np.float32)
    ci = 0
    for t in range(NT):
        w = int(nch[t])
        for c in range(NCORES):
            n = int(counts[c, t])
            col = np.full((w * P,), ZROW, np.int64)
            dl = np.full((w * P,), -1.0, np.float32)
            col[:n] = per_ct_src[(c, t)]
            dl[:n] = per_ct_dl[(c, t)]
            idxT[c, :, ci : ci + w] = col.reshape(w, P).T
            dstl[c, :, ci : ci + w] = dl.reshape(w, P).T
        ci += w

    # per-core dinv, laid out [slot-in-tile, tile]
    dinv_pad = np.zeros(NCORES * SHARD, np.float32)
    for c in range(NCORES):
        dinv_pad[c * SHARD : c * SHARD + OWN] = dinv[c * OWN : (c + 1) * OWN]
    dinvT = dinv_pad.reshape(NCORES, NT, P).transpose(0, 2, 1).copy()  # [NCORES,P,NT]

    return idxT, dstl, dinvT, [int(x) for x in nch], totc


def _build(nch, totc):
    import os
    import concourse.bass as bass
    import concourse.bacc as bacc
    import concourse.mybir as mybir
    import concourse.tile as tile

    f32 = mybir.dt.float32
    bf16 = mybir.dt.bfloat16
    i32 = mybir.dt.int32

    nc = bacc.Bacc("TRN2", target_bir_lowering=False, debug=False,
                   num_devices=NCORES)

    xT = nc.dram_tensor("xT", [2, P, SHARD], f32, kind="ExternalInput")
    idxs = nc.dram_tensor("idxs", [P, totc], i32, kind="ExternalInput")
    dstl = nc.dram_tensor("dstl", [P, totc], f32, kind="ExternalInput")
    dinvT = nc.dram_tensor("dinvT", [P, NT], f32, kind="ExternalInput")
    Ws = nc.dram_tensor("Ws", [L, 2, P, F], f32, kind="ExternalInput")
    brep = nc.dram_tensor("brep", [L, P, F], f32, kind="ExternalInput")
    Wo = nc.dram_tensor("Wo", [2, P, C], f32, kind="ExternalInput")
    borep = nc.dram_tensor("borep", [P, C], f32, kind="ExternalInput")
    iota_in = nc.dram_tensor("iota", [P, P], f32, kind="ExternalInput")
    ident_in = nc.dram_tensor("ident", [P, P], f32, kind="ExternalInput")
    out_o = nc.dram_tensor("out", [SHARD, C], f32, kind="ExternalOutput")
    alpha_o = nc.dram_tensor("alpha", [SHARD, L], f32, kind="ExternalOutput")

    rg = [list(range(NCORES))]

    with tile.TileContext(nc) as tc:
        with tc.tile_pool(name="const", bufs=1) as constp, \
             tc.tile_pool(name="gath", bufs=14) as gathp, \
             tc.tile_pool(name="sel", bufs=8) as selp, \
             tc.tile_pool(name="meta", bufs=4) as metap, \
             tc.tile_pool(name="work", bufs=3) as workp, \
             tc.tile_pool(name="att", bufs=2) as attp, \
             tc.tile_pool(name="psum", bufs=2, space="PSUM") as psump, \
             tc.tile_pool(name="dram", bufs=1, space="DRAM") as dramp:

            # ---- constants into SBUF
            iota_t = constp.tile([P, P], f32)
            nc.sync.dma_start(iota_t[:], iota_in.ap())
            ident_t = constp.tile([P, P], f32)
            nc.sync.dma_start(ident_t[:], ident_in.ap())
            dinv_t = constp.tile([P, NT], f32)
            nc.sync.dma_start(dinv_t[:], dinvT.ap())
            W_t = []
            b_t = []
            for l in range(L):
                w0 = constp.tile([P, F], f32, tag=f"w{l}0")
                w1 = constp.tile([P, F], f32, tag=f"w{l}1")
                nc.sync.dma_start(w0[:], Ws.ap()[l, 0])
                nc.sync.dma_start(w1[:], Ws.ap()[l, 1])
                W_t.append((w0, w1))
                bt = constp.tile([P, F], f32, tag=f"b{l}")
                nc.sync.dma_start(bt[:], brep.ap()[l])
                b_t.append(bt)
            Wo_t = []
            for k in range(2):
                w = constp.tile([P, C], f32, tag=f"wo{k}")
                nc.sync.dma_start(w[:], Wo.ap()[k])
                Wo_t.append(w)
            bo_t = constp.tile([P, C], f32)
            nc.sync.dma_start(bo_t[:], borep.ap())

            # ---- DRAM scratch
            tables = [dramp.tile([NPAD, F], bf16, tag=f"tab{l}", name=f"tab{l}",
                                 addr_space="Shared") for l in range(L)]
            hws_loc = [dramp.tile([SHARD, F], bf16, tag=f"hws{l}",
                                  name=f"hws{l}") for l in range(L)]
            h_bufs = [dramp.tile([SHARD, F], f32, tag=f"h{l}",
                                 name=f"hl{l}") for l in range(L)]

            def transform(lhsT0, lhsT1, l_next, t):
                """hws_{l_next}[t] = (h[t] @ W_{l_next}) * dinv[t]  (bf16)"""
                ps = psump.tile([P, F], f32, tag="tf")
                nc.tensor.matmul(ps[:], lhsT=lhsT0[:], rhs=W_t[l_next][0][:],
                                 start=True, stop=False)
                nc.tensor.matmul(ps[:], lhsT=lhsT1[:], rhs=W_t[l_next][1][:],
                                 start=False, stop=True)
                hws_sb = workp.tile([P, F], bf16, tag="hws_sb")
                nc.vector.tensor_scalar_mul(hws_sb[:], ps[:],
                                            dinv_t[:, t : t + 1])
                nc.sync.dma_start(hws_loc[l_next][t * P : (t + 1) * P, :],
                                  hws_sb[:])

            def transpose_pair(src_sb, tag):
                """[128,256] f32 -> two [128,128] f32 transposed tiles."""
                outs = []
                for k in range(2):
                    pst = psump.tile([P, P], f32, tag=f"tr")
                    nc.tensor.transpose(pst[:], src_sb[:, k * P : (k + 1) * P],
                                        ident_t[:])
                    sb = workp.tile([P, P], f32, tag=f"{tag}{k}")
                    nc.vector.tensor_copy(sb[:], pst[:])
                    outs.append(sb)
                return outs

            # ---- phase 0: hws_0 from x
            for t in range(NT):
                x0 = workp.tile([P, P], f32, tag="x0")
                x1 = workp.tile([P, P], f32, tag="x1")
                nc.sync.dma_start(x0[:], xT.ap()[0, :, t * P : (t + 1) * P])
                nc.sync.dma_start(x1[:], xT.ap()[1, :, t * P : (t + 1) * P])
                transform(x0, x1, 0, t)

            nc.gpsimd.collective_compute(
                "AllGather", mybir.AluOpType.bypass, replica_groups=rg,
                ins=[hws_loc[0][:]], outs=[tables[0][:]])

            phases = int(os.environ.get("K_PHASES", "5"))
            # ---- layers
            ci0 = [0]
            for t in range(NT):
                ci0.append(ci0[-1] + nch[t])

            for l in range(L if phases >= 4 else phases):
                for t in range(NT):
                    w = nch[t]
                    ci = ci0[t]
                    idx_t = metap.tile([P, max(nch)], i32, tag="idx")
                    dst_t = metap.tile([P, max(nch)], f32, tag="dstl")
                    nc.sync.dma_start(idx_t[:, :w], idxs.ap()[:, ci : ci + w])
                    nc.sync.dma_start(dst_t[:, :w], dstl.ap()[:, ci : ci + w])
                    acc = psump.tile([P, F], f32, tag="acc")
                    for c in range(w):
                        gt = gathp.tile([P, F], bf16, tag="gath")
                        nc.gpsimd.indirect_dma_start(
                            out=gt[:], out_offset=None, in_=tables[l][:],
                            in_offset=bass.IndirectOffsetOnAxis(
                                ap=idx_t[:, c : c + 1], axis=0))
                        sel = selp.tile([P, P], bf16, tag="sel")
                        nc.vector.tensor_tensor(
                            out=sel[:],
                            in0=dst_t[:, c : c + 1].to_broadcast([P, P]),
                            in1=iota_t[:], op=mybir.AluOpType.is_equal)
                        nc.tensor.matmul(acc[:], lhsT=sel[:], rhs=gt[:],
                                         start=(c == 0), stop=(c == w - 1))
                    # h = relu(acc * dinv + b)
                    tmp = workp.tile([P, F], f32, tag="tmp")
                    nc.vector.scalar_tensor_tensor(
                        out=tmp[:], in0=acc[:], scalar=dinv_t[:, t : t + 1],
                        in1=b_t[l][:], op0=mybir.AluOpType.mult,
                        op1=mybir.AluOpType.add)
                    h_sb = workp.tile([P, F], f32, tag="h_sb")
                    nc.vector.tensor_scalar_max(h_sb[:], tmp[:], 0.0)
                    nc.sync.dma_start(h_bufs[l][t * P : (t + 1) * P, :], h_sb[:])
                    if l < L - 1:
                        hT = transpose_pair(h_sb, "hT")
                        transform(hT[0], hT[1], l + 1, t)
                if l < L - 1:
                    nc.gpsimd.collective_compute(
                        "AllGather", mybir.AluOpType.bypass, replica_groups=rg,
                        ins=[hws_loc[l + 1][:]], outs=[tables[l + 1][:]])

            # ---- attention readout + output projection
            for t in range(NT if phases >= 5 else 0):
                hs = []
                for l in range(L):
                    ht = attp.tile([P, F], f32, tag=f"ah{l}")
                    nc.sync.dma_start(ht[:], h_bufs[l][t * P : (t + 1) * P, :])
                    hs.append(ht)
                sc = attp.tile([P, L], f32, tag="sc")
                q16 = attp.tile([P, F], f32, tag="q16")
                nc.vector.tensor_scalar_mul(q16[:], hs[L - 1][:], 1.0 / 16.0)
                scratch = attp.tile([P, F], f32, tag="scratch")
                for l in range(L):
                    nc.vector.tensor_tensor(
                        out=scratch[:], in0=hs[l][:], in1=q16[:],
                        op=mybir.AluOpType.mult)
                    nc.vector.tensor_reduce(
                        sc[:, l : l + 1], scratch[:],
                        axis=mybir.AxisListType.X, op=mybir.AluOpType.add)
                m = attp.tile([P, 1], f32, tag="m")
                nc.vector.tensor_reduce(m[:], sc[:], axis=mybir.AxisListType.X,
                                        op=mybir.AluOpType.max)
                negm = attp.tile([P, 1], f32, tag="negm")
                nc.vector.tensor_scalar_mul(negm[:], m[:], -1.0)
                ex = attp.tile([P, L], f32, tag="ex")
                nc.scalar.activation(ex[:], sc[:],
                                     mybir.ActivationFunctionType.Exp,
                                     bias=negm[:, :1], scale=1.0)
                z = attp.tile([P, 1], f32, tag="z")
                nc.vector.tensor_reduce(z[:], ex[:], axis=mybir.AxisListType.X,
                                        op=mybir.AluOpType.add)
                rz = attp.tile([P, 1], f32, tag="rz")
                nc.vector.reciprocal(rz[:], z[:])
                al = attp.tile([P, L], f32, tag="al")
                nc.vector.tensor_scalar_mul(al[:], ex[:], rz[:, :1])
                nc.sync.dma_start(alpha_o.ap()[t * P : (t + 1) * P, :], al[:])
                ho = attp.tile([P, F], f32, tag="ho")
                nc.vector.tensor_scalar_mul(ho[:], hs[0][:], al[:, 0:1])
                for l in range(1, L):
                    nc.vector.scalar_tensor_tensor(
                        out=ho[:], in0=hs[l][:], scalar=al[:, l : l + 1],
                        in1=ho[:], op0=mybir.AluOpType.mult,
                        op1=mybir.AluOpType.add)
                hoT = transpose_pair(ho, "hoT")
                po = psump.tile([P, C], f32, tag="tf")
                nc.tensor.matmul(po[:], lhsT=hoT[0][:], rhs=Wo_t[0][:],
                                 start=True, stop=False)
                nc.tensor.matmul(po[:], lhsT=hoT[1][:], rhs=Wo_t[1][:],
                                 start=False, stop=True)
                ot = attp.tile([P, C], f32, tag="ot")
                nc.vector.tensor_add(ot[:], po[:], bo_t[:])
                nc.sync.dma_start(out_o.ap()[t * P : (t + 1) * P, :], ot[:])

    nc.compile()
    return nc


def kernel(x, edge_index, W0, b0, W1, b1, W2, b2, W3, b3, Wout, bout):
    from concourse.bass_utils import run_bass_kernel_spmd

    x = np.asarray(x, np.float32)
    edge_index = np.asarray(edge_index)
    Wl = [np.asarray(w, np.float32) for w in (W0, W1, W2, W3)]
    bl = [np.asarray(b, np.float32) for b in (b0, b1, b2, b3)]
    Wout = np.asarray(Wout, np.float32)
    bout = np.asarray(bout, np.float32)

    idxT, dstl, dinvT, nch, totc = _host_prep(edge_index)

    key = tuple(nch)
    if key not in _CACHE:
        _CACHE[key] = _build(nch, totc)
    nc = _CACHE[key]

    # shared (replicated) inputs
    Ws_in = np.stack([w.reshape(2, P, F) for w in Wl])  # [L,2,P,F] (K halves)
    brep = np.stack([np.broadcast_to(b, (P, F)) for b in bl]).astype(np.float32)
    Wo_in = Wout.reshape(2, P, C)
    borep = np.broadcast_to(bout, (P, C)).astype(np.float32).copy()
    iota = np.broadcast_to(np.arange(P, dtype=np.float32), (P, P)).copy()
    ident = np.eye(P, dtype=np.float32)

    in_maps = []
    for c in range(NCORES):
        xs = np.zeros((SHARD, F), np.float32)
        xs[:OWN] = x[c * OWN : (c + 1) * OWN]
        xT = xs.T.reshape(2, P, SHARD).copy()
        in_maps.append({
            "xT": xT, "idxs": idxT[c], "dstl": dstl[c], "dinvT": dinvT[c],
            "Ws": Ws_in, "brep": brep, "Wo": Wo_in, "borep": borep,
            "iota": iota, "ident": ident,
        })

    res = run_bass_kernel_spmd(nc, in_maps, core_ids=list(range(NCORES)))
    kernel.last_result = res

    out = np.concatenate(
        [res.results[c]["out"][:OWN] for c in range(NCORES)], axis=0)
    alpha = np.concatenate(
        [res.results[c]["alpha"][:OWN] for c in range(NCORES)], axis=0)
    return out, alpha
